# revision 32
# baseline (speedup 1.0000x reference)
"""Trainium2 Bass kernel for nn_Decoder2 (GRU decoder, Keras reset_after GRUCell).

Reference computation (per batch row b, scanned over t = 0..T-1):
    x_t   = [o_{t-1}, feat_t]                  # [1+F]
    mx    = x_t @ K + ib                       # [3H]
    mh    = h_{t-1} @ Wr + rb                  # [3H]
    z     = sigmoid(mx[:H]   + mh[:H])
    r     = sigmoid(mx[H:2H] + mh[H:2H])
    cand  = tanh(mx[2H:] + r * mh[2H:])
    h_t   = z * h_{t-1} + (1-z) * cand
    o_t   = h_t @ dw + db                      # scalar output per row

Shapes: B=8192, T=96, F=64, H=256.  Data parallel over batch: 1024 rows
per core on 8 cores, no collectives.

v3 design (vs the v2 baseline):
  * o-feedback fully folded off the critical path:
      - z/r gates: Wr' = Wr + dw@k0^T on the z/r columns (host-side fold),
      - h gate (xh): o_{t-1}*k0_h enters the xh PSUM as a K=1 matmul with
        moving operand o_sb(t-1) (computed the step before; a full step of
        slack).  db is folded into activation biases / host-side output add.
    So x tiles carry only the 64 feature rows; since F=64 == 128/2, the
    feature matmuls (K=64) are row-tile PAIRED: two independent K=64
    matmuls run concurrently in the upper/lower halves of the PE array
    (feature rows are duplicated at partitions 64..127).
  * recurrent matmuls in fp8 e4m3 with perf_mode=DoubleRow: one K=256
    matmul per gate chunk (2x tensor-engine throughput vs bf16).  Weights
    are pre-scaled by S=16 so 0.05-scale values stay in e4m3's normal
    range; the 1/S compensation rides the activation engine's free
    `scale` operand.
  * `xh += r*hh` is done on the TENSOR engine as an identity-weight
    matmul accumulating rh into the open xh PSUM group (saves a 1x-rate
    PSUM-source DVE pass).
  * h_new = cand + z*(h - cand) as three 2x-rate bf16 tensor_tensor ops;
    the final add is emitted twice: once with fp8 output (h8, feeds next
    step's DoubleRow matmuls - on the critical chain) and once with bf16
    output (for the dense output / next combine - off-chain, optionally
    on GPSIMD).
  * per-block software pipelining: each (t, j) block carries the chain
    TAIL (identity-ci, tanh, combine) of the previous block, so the PE
    queue never head-of-line blocks on the DVE/ACT chain.

PSUM budget (8 banks): zr pool 2 (r then z sequentially per block),
hh/dense-o pool 2, xh pool 2x2 (double buffered across blocks).
"""

import os
import sys

for _p in ("/root/.axon_site/_ro/trn_rl_repo", "/opt/trn_rl_repo"):
    if os.path.isdir(_p) and _p not in sys.path:
        sys.path.insert(0, _p)

from contextlib import ExitStack  # noqa: E402

import numpy as np  # noqa: E402

import concourse.bacc as bacc  # noqa: E402
import concourse.tile as tile  # noqa: E402
from concourse import mybir  # noqa: E402
from concourse import bass_utils  # noqa: E402

Alu = mybir.AluOpType
Act = mybir.ActivationFunctionType
PerfMode = mybir.MatmulPerfMode

B, T, F, H = 8192, 96, 64, 256
G3 = 3 * H
NCORES = 8
BL = B // NCORES
NCH = 2                      # 128-row chunks of H


def build_nc(
    t_steps: int = T,
    bl: int = BL,
    nt: int = 2,
    fp8: bool = False,
    gps_addbf: bool = False,
    ocast_act: bool = True,
    ci_pe: bool = False,
):
    """Build + compile the per-core Bass program.

    fp8:       recurrent (h) matmuls in e4m3 DoubleRow (else bf16, 2 MMs/chunk)
    gps_addbf: emit the bf16 copy of h_new on GPSIMD (else VectorE).
               NOTE: measured harmful - GPSIMD shares an SBUF port with
               VectorE and knocks every DVE op down to 1x rate.
    ocast_act: dense-o PSUM->SBUF cast on ScalarE (else VectorE)
    ci_pe:     xh += r*hh via identity matmul on TensorE (else in-place DVE
               add).  Only pays when the tensor engine has headroom (fp8).
    """
    n = bl // nt
    assert n <= 512
    f32 = mybir.dt.float32
    bf = mybir.dt.bfloat16
    f8 = mybir.dt.float8e4
    hdt = f8 if fp8 else bf
    inv_s = 1.0 / 16.0 if fp8 else 1.0   # weights are pre-scaled by 1/inv_s

    nc = bacc.Bacc("TRN2", target_bir_lowering=False, debug=False)

    featT2 = nc.dram_tensor("featT2", [t_steps, 128, bl], bf, kind="ExternalInput").ap()
    h0T = nc.dram_tensor("h0T", [128, NCH, bl], bf, kind="ExternalInput").ap()
    h08T = nc.dram_tensor("h08T", [128, NCH, bl], hdt, kind="ExternalInput").ap()
    o0 = nc.dram_tensor("o0", [1, bl], bf, kind="ExternalInput").ap()
    # x-side weights, pre-scaled by S: rows 0..63 = kernel[1:], rows 64..127 dup
    kxf = nc.dram_tensor("kxf", [128, G3], bf, kind="ExternalInput").ap()
    # k0 row replicated at partitions 0 and 32 (K=1 o-matmuls for j=0 / j=1)
    k0t = nc.dram_tensor("k0t", [33, G3], bf, kind="ExternalInput").ap()
    # recurrent weights (folded z/r cols), [ki, ko, col] layout, pre-scaled
    wrq = nc.dram_tensor("wrq", [128, NCH, G3], hdt, kind="ExternalInput").ap()
    # unfolded z/r columns for t=0 (o_{-1} is the external init input)
    wrz0q = nc.dram_tensor("wrz0q", [128, NCH, 2 * H], hdt, kind="ExternalInput").ap()
    dww = nc.dram_tensor("dww", [128, NCH], bf, kind="ExternalInput").ap()
    ident = nc.dram_tensor("ident", [128, 128], bf, kind="ExternalInput").ap()
    outT = nc.dram_tensor("outT", [t_steps, bl], bf, kind="ExternalOutput").ap()

    with tile.TileContext(nc) as tc, ExitStack() as ctx:
        const = ctx.enter_context(tc.tile_pool(name="const", bufs=1))
        hpool = ctx.enter_context(tc.tile_pool(name="h", bufs=3))
        h8pool = ctx.enter_context(tc.tile_pool(name="h8", bufs=3))
        xpool = ctx.enter_context(tc.tile_pool(name="x", bufs=6))
        rpool = ctx.enter_context(tc.tile_pool(name="rsb", bufs=3))
        zpool = ctx.enter_context(tc.tile_pool(name="zsb", bufs=3))
        rhpool = ctx.enter_context(tc.tile_pool(name="rh", bufs=3))
        cpool = ctx.enter_context(tc.tile_pool(name="cand", bufs=3))
        dpool = ctx.enter_context(tc.tile_pool(name="dsb", bufs=3))
        epool = ctx.enter_context(tc.tile_pool(name="esb", bufs=3))
        opool = ctx.enter_context(tc.tile_pool(name="osb", bufs=4))
        pzr = ctx.enter_context(tc.tile_pool(name="pzr", bufs=1, space="PSUM"))
        phh = ctx.enter_context(tc.tile_pool(name="phh", bufs=1, space="PSUM"))
        pxh = ctx.enter_context(tc.tile_pool(name="pxh", bufs=2, space="PSUM"))

        # --- constants ---
        kxf_sb = const.tile([128, G3], bf)
        nc.sync.dma_start(out=kxf_sb, in_=kxf)
        k0t_sb = const.tile([33, G3], bf)
        nc.sync.dma_start(out=k0t_sb, in_=k0t)
        wr_sb = const.tile([128, NCH, G3], hdt)
        nc.sync.dma_start(out=wr_sb, in_=wrq)
        wrz0_sb = const.tile([128, NCH, 2 * H], hdt)
        nc.sync.dma_start(out=wrz0_sb, in_=wrz0q)
        dw_sb = const.tile([128, NCH], bf)
        nc.sync.dma_start(out=dw_sb, in_=dww)
        id_sb = const.tile([128, 128], bf)
        nc.sync.dma_start(out=id_sb, in_=ident)
        # o rows live at partition 0 (j=0) and 32 (j=1) so the K=1 o-matmuls
        # hit distinct PE row groups
        o0_sb = const.tile([33, bl], bf)
        nc.sync.dma_start(out=o0_sb[0:1, :], in_=o0)
        nc.sync.dma_start(out=o0_sb[32:33, :], in_=o0)

        # --- initial state ---
        h_prev = hpool.tile([128, NCH, bl], bf)
        nc.sync.dma_start(out=h_prev, in_=h0T)
        h8_prev = h8pool.tile([128, NCH, bl], hdt)
        nc.sync.dma_start(out=h8_prev, in_=h08T)
        xs = {}
        for j in range(nt):
            xj = xpool.tile([128, n], bf, tag="x")
            nc.sync.dma_start(out=xj, in_=featT2[0, :, j * n:(j + 1) * n])
            xs[(0, j)] = xj

        def h_mms(gp, gcol, t, j, bs, ncol=128):
            """Recurrent matmuls for one 128-wide gate chunk starting at
            column gcol.  gp: PSUM target [128, n].  Never the group opener
            (feat matmuls start the group); closes the group unless t==0
            (the K=1 o0 matmul closes it then)."""
            w = wrz0_sb if (t == 0 and gcol < 2 * H) else wr_sb
            if fp8:
                nc.tensor.matmul(gp, w[:, :, gcol:gcol + ncol],
                                 h8_prev[:, :, bs], start=False, stop=t != 0,
                                 perf_mode=PerfMode.DoubleRow)
            else:
                nc.tensor.matmul(gp, w[:, 0, gcol:gcol + ncol],
                                 h8_prev[:, 0, bs], start=False, stop=False)
                nc.tensor.matmul(gp, w[:, 1, gcol:gcol + ncol],
                                 h8_prev[:, 1, bs], start=False, stop=t != 0)
            if t == 0:
                nc.tensor.matmul(gp, k0t_sb[0:1, gcol:gcol + ncol],
                                 o0_sb[0:1, bs], start=False, stop=True)

        # state carried between blocks for the deferred chain tail
        pending = None        # set by the main loop after rh
        pending_head = None   # set by emit_tail_head, consumed by _combine

        def emit_tail_head():
            """First half of the previous block's chain tail: ci + tanh.
            Emitted early so the DVE's ci heads its queue."""
            nonlocal pending, pending_head
            if pending is None:
                return
            pt, pj, pbs, xhp, rh_sb, zsb, hprv, hnew, h8new = pending
            pending = None
            if ci_pe:
                # xh += rh via identity matmul (closes the xh group)
                nc.tensor.matmul(xhp[:, 0, :], id_sb, rh_sb[:, 0, :],
                                 start=False, stop=True)
                nc.tensor.matmul(xhp[:, 1, :], id_sb, rh_sb[:, 1, :],
                                 start=False, stop=True)
            else:
                nc.vector.tensor_tensor(xhp, xhp, rh_sb, Alu.add)
            cand = cpool.tile([128, NCH, n], bf, tag="cand")
            nc.scalar.activation(cand, xhp, Act.Tanh, scale=inv_s)
            pending_head = (pbs, cand, zsb, hprv, hnew, h8new)

        def emit_tail_combine():
            """Second half: h_new = cand + z*(h_prev - cand).  Emitted after
            this block's rh so rh sits early in the DVE queue (frees the
            phh bank for the next block's dense-o)."""
            nonlocal pending_head
            if pending_head is None:
                return
            pbs, cand, zsb, hprv, hnew, h8new = pending_head
            pending_head = None
            d_sb = dpool.tile([128, NCH, n], bf, tag="dsb")
            nc.vector.tensor_tensor(d_sb, hprv[:, :, pbs], cand, Alu.subtract)
            e_sb = epool.tile([128, NCH, n], bf, tag="esb")
            nc.vector.tensor_tensor(e_sb, zsb, d_sb, Alu.mult)
            # fp8 copy first (critical chain: feeds next step's matmuls)
            if fp8:
                nc.vector.tensor_tensor(h8new[:, :, pbs], cand, e_sb, Alu.add)
            if gps_addbf:
                nc.gpsimd.tensor_tensor(hnew[:, :, pbs], cand, e_sb, Alu.add)
            else:
                nc.vector.tensor_tensor(hnew[:, :, pbs], cand, e_sb, Alu.add)

        def emit_tail():
            emit_tail_head()
            emit_tail_combine()

        os_sb = {}   # (t, j) -> o_sb tile [1, n] (dense output, bf16, o - db)

        def emit_dense_o(t, j, h_t):
            """Dense output o(t, j) = h(t, j-half) @ dw."""
            bs = slice(j * n, (j + 1) * n)
            po = phh.tile([1, n], f32, tag="phh")
            nc.tensor.matmul(po, dw_sb[:, 0:1], h_t[:, 0, bs],
                             start=True, stop=False)
            nc.tensor.matmul(po, dw_sb[:, 1:2], h_t[:, 1, bs],
                             start=False, stop=True)
            o_sb = opool.tile([33, n], bf, tag="osb")
            if ocast_act:
                nc.scalar.activation(o_sb[0:1, :], po, Act.Copy)
            else:
                nc.vector.tensor_copy(out=o_sb[0:1, :], in_=po)
            # replicate at partition 32 (free: DMA) so the two K=1 o-matmuls
            # of the next step can row-tile pair
            nc.sync.dma_start(out=o_sb[32:33, :], in_=o_sb[0:1, :])
            nc.sync.dma_start(out=outT[t:t + 1, bs], in_=o_sb[0:1, :])
            os_sb[(t, j)] = o_sb

        h_hist = {-1: (h_prev, h8_prev)}
        for t in range(t_steps):
            h_new = hpool.tile([128, NCH, bl], bf, tag="h")
            if fp8:
                h8_new = h8pool.tile([128, NCH, bl], hdt, tag="h8")
            else:
                h8_new = h_new
            h_hist[t] = (h_new, h8_new)
            h_prev, h8_prev = h_hist[t - 1]
            for j in range(nt):
                bs = slice(j * n, (j + 1) * n)
                x = xs[(t, j)]

                # --- r + xh feature matmuls, row-tile paired (c0 hi / c1 lo) ---
                rp = pzr.tile([128, NCH, n], f32, tag="pzr")
                nc.tensor.matmul(rp[:, 0, :], kxf_sb[64:128, H:H + 128],
                                 x[64:128, :], start=True, stop=False)
                nc.tensor.matmul(rp[:, 1, :], kxf_sb[0:64, H + 128:2 * H],
                                 x[0:64, :], start=True, stop=False)
                xhp = pxh.tile([128, NCH, n], f32, tag="pxh")
                nc.tensor.matmul(xhp[:, 0, :], kxf_sb[64:128, 2 * H:2 * H + 128],
                                 x[64:128, :], start=True, stop=False)
                nc.tensor.matmul(xhp[:, 1, :], kxf_sb[0:64, 2 * H + 128:G3],
                                 x[0:64, :], start=True, stop=False)

                # --- dense output + o-cast of step t-1 (slack: one step).
                # Early so the ocast heads the ACT queue (its po matmuls run
                # first on the PE) and frees the phh slot before hh ---
                if t > 0:
                    emit_dense_o(t - 1, j, h_prev)

                # --- r recurrent matmuls ---
                h_mms(rp[:, 0, :], H, t, j, bs)
                h_mms(rp[:, 1, :], H + 128, t, j, bs)
                r_sb = rpool.tile([128, NCH, n], bf, tag="rsb")
                nc.scalar.activation(r_sb, rp, Act.Sigmoid, scale=inv_s)

                # --- previous block's chain tail (first half) ---
                emit_tail_head()

                # --- z matmuls (zr PSUM slot reused after sig_r read) ---
                zp = pzr.tile([128, NCH, n], f32, tag="pzr")
                nc.tensor.matmul(zp[:, 0, :], kxf_sb[64:128, 0:128],
                                 x[64:128, :], start=True, stop=False)
                nc.tensor.matmul(zp[:, 1, :], kxf_sb[0:64, 128:256],
                                 x[0:64, :], start=True, stop=False)
                h_mms(zp[:, 0, :], 0, t, j, bs)
                h_mms(zp[:, 1, :], 128, t, j, bs)
                z_sb = zpool.tile([128, NCH, n], bf, tag="zsb")
                nc.scalar.activation(z_sb, zp, Act.Sigmoid, scale=inv_s)

                # --- hh matmuls ---
                hhp = phh.tile([128, NCH, n], f32, tag="phh")
                if fp8:
                    nc.tensor.matmul(hhp[:, 0, :], wr_sb[:, :, 2 * H:2 * H + 128],
                                     h8_prev[:, :, bs], start=True, stop=True,
                                     perf_mode=PerfMode.DoubleRow)
                    nc.tensor.matmul(hhp[:, 1, :], wr_sb[:, :, 2 * H + 128:G3],
                                     h8_prev[:, :, bs], start=True, stop=True,
                                     perf_mode=PerfMode.DoubleRow)
                else:
                    for c in range(NCH):
                        m = 2 * H + c * 128
                        nc.tensor.matmul(hhp[:, c, :], wr_sb[:, 0, m:m + 128],
                                         h8_prev[:, 0, bs], start=True, stop=False)
                        nc.tensor.matmul(hhp[:, c, :], wr_sb[:, 1, m:m + 128],
                                         h8_prev[:, 1, bs], start=False, stop=True)

                # --- xh o-feedback: K=1 matmuls with o(t-1), row-tile
                # paired via the o copies at partitions 0 and 32 ---
                if t == 0:
                    o0p = o0_sb
                    c0s, c1s = slice(0, 1), slice(32, 33)
                    o_p0, o_p1 = o0p[c0s, bs], o0p[c1s, bs]
                else:
                    osb = os_sb[(t - 1, j)]
                    o_p0, o_p1 = osb[0:1, :], osb[32:33, :]
                nc.tensor.matmul(xhp[:, 0, :], k0t_sb[0:1, 2 * H:2 * H + 128],
                                 o_p0, start=False, stop=not ci_pe)
                nc.tensor.matmul(xhp[:, 1, :], k0t_sb[32:33, 2 * H + 128:G3],
                                 o_p1, start=False, stop=not ci_pe)

                # --- rh = hh * r  (early in the DVE queue; the combine of
                # the previous block is emitted after it) ---
                rh_sb = rhpool.tile([128, NCH, n], bf, tag="rh")
                nc.vector.tensor_tensor(rh_sb, hhp, r_sb, Alu.mult)

                emit_tail_combine()

                pending = (t, j, bs, xhp, rh_sb, z_sb, h_prev, h_new, h8_new)

                # --- prefetch next step's features ---
                if t < t_steps - 1:
                    xj = xpool.tile([128, n], bf, tag="x")
                    nc.sync.dma_start(
                        out=xj, in_=featT2[t + 1, :, j * n:(j + 1) * n])
                    xs[(t + 1, j)] = xj

            h_hist.pop(t - 2, None)
            xs.pop((t - 1, 0), None)
            xs.pop((t - 1, 1), None)
            for jj in range(nt):
                os_sb.pop((t - 2, jj), None)

        # final chain tail + last step's dense outputs
        emit_tail()
        for j in range(nt):
            emit_dense_o(t_steps - 1, j, h_hist[t_steps - 1][0])

    nc.compile()
    return nc


_NC_CACHE: dict = {}


def _flags():
    return dict(
        fp8=os.environ.get("V3_FP8", "0") == "1",
        gps_addbf=os.environ.get("V3_GPS", "0") == "1",
        ocast_act=os.environ.get("V3_OCAST_ACT", "1") == "1",
        ci_pe=os.environ.get("V3_CI_PE", "0") == "1",
    )


def _get_nc(t_steps=T, bl=BL, nt=2, **kw):
    flags = {**_flags(), **kw}
    key = (t_steps, bl, nt, tuple(sorted(flags.items())))
    if key not in _NC_CACHE:
        _NC_CACHE[key] = build_nc(t_steps, bl, nt, **flags)
    return _NC_CACHE[key]


def make_in_maps(
    decoder_feature, init_state, decoder_init_input, kernel, recurrent_kernel,
    input_bias, recurrent_bias, dense_w, dense_b,
    fp8=True, t_steps=T, bl=BL, ncores=NCORES,
):
    bf_np = mybir.dt.np(mybir.dt.bfloat16)
    h_np = mybir.dt.np(mybir.dt.float8e4) if fp8 else bf_np
    S = 16.0 if fp8 else 1.0

    f = np.asarray(decoder_feature, np.float32)
    h0 = np.asarray(init_state, np.float32)
    o0 = np.asarray(decoder_init_input, np.float32)
    kx = np.asarray(kernel, np.float32)
    wr = np.asarray(recurrent_kernel, np.float32)
    ib = np.asarray(input_bias, np.float32)
    rb = np.asarray(recurrent_bias, np.float32)
    dw = np.asarray(dense_w, np.float32)
    db = float(np.asarray(dense_b, np.float32).reshape(-1)[0])
    assert not ib.any() and not rb.any(), \
        "nonzero GRU biases not supported by this kernel variant"
    k0 = kx[0]

    # o-feedback fold into the recurrent weights (z/r columns):
    #   o_{t-1} = h_{t-1} @ dw + db ;  the db part goes nowhere (db == 0
    #   required unless biases are folded -- assert instead).
    assert db == 0.0 or True  # db handled exactly via o_raw & host-side add
    wr_folded = wr.copy()
    wr_folded[:, :2 * H] += dw @ k0[None, :2 * H]
    # db*k0 constant term: with zero ib/rb the only bias on the gates is
    # db*k0 (o includes db).  o_raw rides the K=1 matmul without db, so the
    # db*k0 term must be constant-folded; require db == 0 (true for this
    # problem) to keep the bias-free fast path.
    assert db == 0.0, "nonzero dense bias needs the bias path"

    kxf = np.empty((128, G3), np.float32)
    kxf[0:64] = kx[1:] * S
    kxf[64:128] = kx[1:] * S
    k0t = np.zeros((33, G3), np.float32)
    k0t[0] = k0 * S
    k0t[32] = k0 * S

    def to_h(a):
        return np.ascontiguousarray(a).astype(h_np)

    wrq = (wr_folded * S).reshape(2, 128, G3).transpose(1, 0, 2)
    wrz0q = (wr[:, :2 * H] * S).reshape(2, 128, 2 * H).transpose(1, 0, 2)

    in_maps = []
    for i in range(ncores):
        s = slice(i * bl, (i + 1) * bl)
        featT = f[s, :t_steps].transpose(1, 2, 0)      # [T, F, bl]
        featT2 = np.concatenate([featT, featT], axis=1)  # [T, 128, bl]
        h0T = h0[s].T.reshape(2, 128, bl).transpose(1, 0, 2)
        in_maps.append({
            "featT2": np.ascontiguousarray(featT2).astype(bf_np),
            "h0T": np.ascontiguousarray(h0T).astype(bf_np),
            "h08T": to_h(h0T),
            "o0": np.ascontiguousarray((o0[s] - db).T).astype(bf_np),
            "kxf": kxf.astype(bf_np),
            "k0t": np.ascontiguousarray(k0t).astype(bf_np),
            "wrq": to_h(wrq),
            "wrz0q": to_h(wrz0q),
            "dww": np.ascontiguousarray(dw.reshape(2, 128).T).astype(bf_np),
            "ident": np.eye(128, dtype=np.float32).astype(bf_np),
        })
    return in_maps, db


def run(inputs: dict, nt=2, trace=False, trace_kwargs=None, **kw):
    t_steps = int(inputs.get("predict_seq_length", T))
    assert t_steps == T, f"kernel hardcodes T={T}, got {t_steps}"
    flags = {**_flags(), **kw}
    nc = _get_nc(T, BL, nt, **flags)
    in_maps, db = make_in_maps(
        inputs["decoder_feature"], inputs["init_state"],
        inputs["decoder_init_input"], inputs["kernel"],
        inputs["recurrent_kernel"], inputs["input_bias"],
        inputs["recurrent_bias"], inputs["dense_w"], inputs["dense_b"],
        fp8=flags["fp8"],
    )
    res = bass_utils.run_bass_kernel_spmd(
        nc, in_maps, core_ids=list(range(NCORES)), trace=trace,
        **(trace_kwargs or {}),
    )
    out = np.empty((B, T, 1), np.float32)
    for i in range(NCORES):
        out[i * BL:(i + 1) * BL, :, 0] = \
            res.results[i]["outT"].astype(np.float32).T + db
    return out, res


def kernel(**inputs) -> np.ndarray:
    out, _ = run(inputs)
    return out
